# revision 11
# baseline (speedup 1.0000x reference)
"""Trainium2 Bass kernel for nn_Cross_attention_3 (sparse_attention).

Sharding: the (D, H*W) plane is unfolded into 9x9 patches; D=72 gives 8
patch-row blocks of 9 rows — exactly one per NeuronCore.  The only
cross-core dependency is the AdaptiveAvgPool over the patch axis
(bins of 128 patches straddle core boundaries); cores exchange 16-patch
sub-block partial sums (gcd(128, 1296) = 16) via a 1.7MB AllGather.

The two MLP linears have no nonlinearity between them, so they collapse
into a single 81x81 matrix; the conv bias rides along as an 82nd
contraction row whose rhs holds b[c].  The 1x1x1 conv is computed with
the patch data as the matmul's stationary operand, so its output lands
directly in (patch-element, channel) layout — the transpose the rest of
the pipeline needs comes for free.
"""

import os
import sys

import numpy as np

try:
    import concourse.bacc as _  # noqa: F401
except ImportError:  # container default path
    sys.path.insert(0, "/opt/trn_rl_repo")

import concourse.bacc as bacc
import concourse.mybir as mybir
from concourse.bass_utils import run_bass_kernel_spmd
from concourse.tile import TileContext

P = 9
P2 = 81
C = 64
D = 72
H = W = 108
HW = H * W
NCORES = 8
LLOC = (HW // P) * 1  # patches per core per D-block row group = 1296
LP = LLOC * P // P // 2  # 648 patch pairs per core
assert HW // P == 1296 and LP == 648

NLP_A = 24   # pass-A chunk, in pairs (48 patches); multiple of 8 for pooling
NCH_A = LP // NLP_A          # 27 chunks
RING_B = 216                 # pass-B ring, in pairs (432 patches)
SUB_B = 24                   # pass-B conv/mlp subchunk, in pairs
NRING = LP // RING_B         # 3
NSUB = RING_B // SUB_B       # 9

F32 = mybir.dt.float32
F32R = mybir.dt.float32r

_cache = {}


def _r(ap):
    return ap


def _build_nc():
    nc = bacc.Bacc(None, target_bir_lowering=False, debug=False)
    xp_d = nc.declare_dram_parameter("xp", [128, LP, P2], F32R, isOutput=False)
    yp_d = nc.declare_dram_parameter("yp", [128, LP, P2], F32R, isOutput=False)
    wi_d = nc.declare_dram_parameter("wi", [128, 256], F32R, isOutput=False)
    wf_d = nc.declare_dram_parameter("wf", [128, 256], F32R, isOutput=False)
    wm_d = nc.declare_dram_parameter("wm", [82, P2], F32R, isOutput=False)
    bi_d = nc.declare_dram_parameter("bi", [1, NLP_A * 128], F32R, isOutput=False)
    bf_d = nc.declare_dram_parameter("bf", [1, NLP_A * 128], F32R, isOutput=False)
    out_d = nc.declare_dram_parameter("out", [P2, C, 2 * LP], F32, isOutput=True)

    gath_d = nc.dram_tensor("gath", [NCORES, P2, C, P2], F32, addr_space="Shared")

    with nc.allow_low_precision("fp32r compute pipeline"), TileContext(nc) as tc:
        with (
            tc.tile_pool(name="const", bufs=1) as constp,
            tc.tile_pool(name="stage", bufs=3) as stagep,
            tc.tile_pool(name="mlps", bufs=2) as mlpp,
            tc.tile_pool(name="psconv", bufs=4, space="PSUM") as psconv,
            tc.tile_pool(name="psmlp", bufs=2, space="PSUM") as psmlp,
            tc.tile_pool(name="dram", bufs=1, space="DRAM") as dramp,
        ):
            wi_sb = constp.tile([128, 256], F32R, tag="wi")
            wf_sb = constp.tile([128, 256], F32R, tag="wf")
            wm_sb = constp.tile([82, P2], F32R, tag="wm")
            pooled = constp.tile([P2, C, P2], F32R, tag="pooled")
            s_dram = dramp.tile([P2, C, P2], F32)
            nc.sync.dma_start(out=wi_sb[:, :], in_=wi_d[:, :])
            nc.sync.dma_start(out=wf_sb[:, :], in_=wf_d[:, :])
            nc.sync.dma_start(out=wm_sb[:, :], in_=wm_d[:, :])

            def conv_mlp(src_d, w_sb, b_d, nlp, lp0, dst, dst_lp0):
                """conv+MLP+lrelu for `nlp` pairs starting at pair lp0 of
                src_d; writes (81, nlp, 128) into dst[:, dst_lp0:, :]."""
                st = stagep.tile([128, SUB_B, P2], F32R, tag="stage")
                nc.sync.dma_start(out=st[:, 0:nlp, :], in_=src_d[:, lp0:lp0 + nlp, :])
                ms = mlpp.tile([82, SUB_B, 128], F32R, tag="ms")
                nc.sync.dma_start(
                    out=ms[81:82, 0:nlp, :],
                    in_=b_d[:, 0:nlp * 128],
                )
                # conv: 2 pairs per PSUM bank; output (81, slot*64+c) per pair
                for pp in range(nlp // 2):
                    ps = psconv.tile([P2, 512], F32, tag="psc")
                    nc.tensor.matmul(
                        ps[0:P2, 0:256], _r(st[:, 2 * pp, :]), _r(w_sb[:, :]),
                        start=True, stop=True,
                    )
                    nc.tensor.matmul(
                        ps[0:P2, 256:512], _r(st[:, 2 * pp + 1, :]), _r(w_sb[:, :]),
                        start=True, stop=True,
                    )
                    # cols [0:128] and [256:384] hold the two pairs' results
                    src = ps[0:P2, 0:512].rearrange("p (a b) -> p a b", a=2)[:, :, 0:128]
                    d = ms[0:P2, 2 * pp:2 * pp + 2, :]
                    nc.vector.tensor_copy(d, src)
                # MLP (+bias row) and LeakyReLU
                ncols = nlp * 128
                flat = ms[:, 0:nlp, :].rearrange("p a b -> p (a b)")
                for q in range(ncols // 512):
                    mp = psmlp.tile([P2, 512], F32, tag="psm")
                    nc.tensor.matmul(
                        mp[0:P2, :], _r(wm_sb[:, :]), _r(flat[:, 512 * q:512 * (q + 1)]),
                        start=True, stop=True,
                    )
                    dcols = dst[0:P2, dst_lp0:dst_lp0 + nlp, :].rearrange(
                        "p a b -> p (a b)"
                    )[:, 512 * q:512 * (q + 1)]
                    nc.scalar.activation(
                        dcols, mp[0:P2, :],
                        mybir.ActivationFunctionType.Prelu, alpha=0.2,
                    )

            # ---------------- pass A: fea (y) + pooling partial sums --------
            with (
                tc.tile_pool(name="feaout", bufs=2) as feap,
                tc.tile_pool(name="ssbp", bufs=1) as ssbp,
                tc.tile_pool(name="comb", bufs=2) as combp,
            ):
                s_sb = ssbp.tile([P2, C, P2], F32, tag="ssb")
                for ch in range(NCH_A):
                    fea = feap.tile([P2, NLP_A, 128], F32, tag="fea")
                    conv_mlp(yp_d, wf_sb, bf_d, NLP_A, ch * NLP_A, fea, 0)
                    for g in range(NLP_A // 8):
                        sidx = ch * (NLP_A // 8) + g
                        nc.vector.tensor_reduce(
                            s_sb[0:P2, :, sidx:sidx + 1],
                            fea[0:P2, 8 * g:8 * (g + 1), :].rearrange(
                                "p l (s c) -> p c l s", s=2
                            ),
                            mybir.AxisListType.XY,
                            mybir.AluOpType.add,
                        )
                nc.gpsimd.dma_start(out=s_dram[:, :, :], in_=s_sb[:, :, :])
                nc.gpsimd.collective_compute(
                    "AllGather",
                    mybir.AluOpType.bypass,
                    replica_groups=[list(range(NCORES))],
                    ins=[s_dram[:, :, :]],
                    outs=[gath_d[:, :, :, :]],
                )
                # combine 648 global 16-sub-blocks into 81 bins of 8
                for cc in range(8):
                    tcb = combp.tile([P2, 8, NCORES * P2], F32, tag="tcb")
                    for k in range(NCORES):
                        nc.gpsimd.dma_start(
                            out=tcb[0:P2, :, P2 * k:P2 * (k + 1)],
                            in_=gath_d[k, :, 8 * cc:8 * (cc + 1), :],
                        )
                    pr = combp.tile([P2, 8, P2], F32, tag="pr")
                    nc.vector.tensor_reduce(
                        pr[0:P2, :, :],
                        tcb[0:P2, :, :].rearrange("p c (j m) -> p c j m", m=8),
                        mybir.AxisListType.X,
                        mybir.AluOpType.add,
                    )
                    nc.vector.tensor_scalar_mul(
                        pooled[0:P2, 8 * cc:8 * (cc + 1), :], pr[0:P2, :, :],
                        1.0 / 128.0,
                    )

            # ---------------- pass B: img (x) + attention -------------------
            with (
                tc.tile_pool(name="imgring", bufs=1) as imgp,
                tc.tile_pool(name="attev", bufs=4) as attevp,
                tc.tile_pool(name="psatt", bufs=2, space="PSUM") as psatt,
            ):
                for ring in range(NRING):
                    img = imgp.tile([P2, RING_B, 128], F32R, tag="img")
                    for sub in range(NSUB):
                        conv_mlp(
                            xp_d, wi_sb, bi_d, SUB_B,
                            ring * RING_B + sub * SUB_B, img, sub * SUB_B,
                        )
                    rhs = img[0:P2, :, :].rearrange("p l (s c) -> p c l s", s=2)
                    l0 = ring * RING_B * 2
                    ncols = RING_B * 2
                    for c in range(C):
                        ap = psatt.tile([P2, 512], F32, tag="psa")
                        nc.tensor.matmul(
                            ap[0:P2, 0:ncols],
                            _r(pooled[:, c:c + 1, :]),
                            _r(rhs[:, c:c + 1, :, :]),
                            start=True, stop=True,
                        )
                        ev = attevp.tile([P2, RING_B * 2], F32, tag="attev")
                        nc.scalar.copy(ev[0:P2, :], ap[0:P2, 0:ncols])
                        nc.sync.dma_start(
                            out=out_d[0:P2, c:c + 1, l0:l0 + ncols], in_=ev[0:P2, :]
                        )
    nc.compile()
    return nc


def _host_prep(x, y, w_img, b_img, w_fea, b_fea, w1, w2):
    f32 = np.float32
    weff = (w2.astype(np.float64) @ w1.astype(np.float64))  # (81, 81)
    wm = np.concatenate([weff.T, weff.sum(axis=1)[None, :]], axis=0).astype(f32)

    def pairw(w):
        blk = np.zeros((128, 128), dtype=f32)
        blk[0:64, 0:64] = w.T
        blk[64:128, 64:128] = w.T
        return np.concatenate([blk, blk], axis=1)

    wi = pairw(w_img.astype(f32))
    wf = pairw(w_fea.astype(f32))
    bi = np.tile(np.concatenate([b_img, b_img]).astype(f32), NLP_A)[None, :]
    bf = np.tile(np.concatenate([b_fea, b_fea]).astype(f32), NLP_A)[None, :]

    def unf_pairs(t):  # t: (1, 64, 72, 108, 108) -> list of (128, 648, 81)
        u = np.ascontiguousarray(
            t.reshape(C, NCORES, P, HW // P, P).transpose(1, 0, 3, 2, 4)
        )  # (8, 64, 1296, 9, 9) -> per core u[k]: (64, 1296, 81)
        u = u.reshape(NCORES, C, HW // P, P2)
        out = []
        for k in range(NCORES):
            v = u[k].reshape(C, LP, 2, P2).transpose(2, 0, 1, 3)  # (2, 64, 648, 81)
            out.append(np.ascontiguousarray(v.reshape(128, LP, P2)))
        return out

    xps = unf_pairs(np.asarray(x, dtype=f32))
    yps = unf_pairs(np.asarray(y, dtype=f32))
    shared = {"wi": wi, "wf": wf, "wm": wm, "bi": bi, "bf": bf}
    return [dict(shared, xp=xps[k], yp=yps[k]) for k in range(NCORES)]


def kernel(x, y, w_img, b_img, w_fea, b_fea, w1, w2):
    if "nc" not in _cache:
        _cache["nc"] = _build_nc()
    nc = _cache["nc"]
    in_maps = _host_prep(x, y, w_img, b_img, w_fea, b_fea, w1, w2)
    trace = bool(os.environ.get("KERNEL_TRACE"))
    res = run_bass_kernel_spmd(
        nc, in_maps, list(range(NCORES)), trace=trace
    )
    _cache["last_result"] = res
    out = np.empty((1, C, D, H, W), dtype=np.float32)
    ov = out.reshape(C, D, HW)
    for k in range(NCORES):
        att = res.results[k]["out"].transpose(1, 2, 0)  # (64, 1296, 81)
        blk = att.reshape(C, HW // P, P, P).transpose(0, 2, 1, 3).reshape(C, P, HW)
        ov[:, P * k:P * (k + 1), :] = blk
    return out


# revision 12
# speedup vs baseline: 1.1003x; 1.1003x over previous
"""Trainium2 Bass kernel for nn_Cross_attention_3 (sparse_attention).

Sharding: the (D, H*W) plane is unfolded into 9x9 patches; D=72 gives 8
patch-row blocks of 9 rows — exactly one per NeuronCore.  The only
cross-core dependency is the AdaptiveAvgPool over the patch axis
(bins of 128 patches straddle core boundaries); cores exchange 16-patch
sub-block partial sums (gcd(128, 1296) = 16) via a 1.7MB AllGather.

The two MLP linears have no nonlinearity between them, so they collapse
into a single 81x81 matrix; the conv bias rides along as an 82nd
contraction row whose rhs holds b[c].  The 1x1x1 conv is computed with
the patch data as the matmul's stationary operand, so its output lands
directly in (patch-element, channel) layout — the transpose the rest of
the pipeline needs comes for free.
"""

import os
import sys

import numpy as np

try:
    import ml_dtypes
except ImportError:
    ml_dtypes = None

try:
    import concourse.bacc as _  # noqa: F401
except ImportError:  # container default path
    sys.path.insert(0, "/opt/trn_rl_repo")

import concourse.bacc as bacc
import concourse.mybir as mybir
from concourse.bass_utils import run_bass_kernel_spmd
from concourse.tile import TileContext

P = 9
P2 = 81
C = 64
D = 72
H = W = 108
HW = H * W
NCORES = 8
LLOC = (HW // P) * 1  # patches per core per D-block row group = 1296
LP = LLOC * P // P // 2  # 648 patch pairs per core
assert HW // P == 1296 and LP == 648

NLP_A = 24   # pass-A chunk, in pairs (48 patches); multiple of 8 for pooling
NCH_A = LP // NLP_A          # 27 chunks
RING_B = 216                 # pass-B ring, in pairs (432 patches)
SUB_B = 24                   # pass-B conv/mlp subchunk, in pairs
NRING = LP // RING_B         # 3
NSUB = RING_B // SUB_B       # 9

F32 = mybir.dt.float32
BF16 = mybir.dt.bfloat16

_cache = {}


def _r(ap):
    return ap


def _build_nc():
    nc = bacc.Bacc(None, target_bir_lowering=False, debug=False)
    xp_d = nc.declare_dram_parameter("xp", [128, LP, P2], BF16, isOutput=False)
    yp_d = nc.declare_dram_parameter("yp", [128, LP, P2], BF16, isOutput=False)
    wi_d = nc.declare_dram_parameter("wi", [128, 256], BF16, isOutput=False)
    wf_d = nc.declare_dram_parameter("wf", [128, 256], BF16, isOutput=False)
    wm_d = nc.declare_dram_parameter("wm", [82, P2], BF16, isOutput=False)
    bi_d = nc.declare_dram_parameter("bi", [1, NLP_A * 128], BF16, isOutput=False)
    bf_d = nc.declare_dram_parameter("bf", [1, NLP_A * 128], BF16, isOutput=False)
    out_d = nc.declare_dram_parameter("out", [P2, C, 2 * LP], F32, isOutput=True)

    gath_d = nc.dram_tensor("gath", [NCORES, P2, C, P2], F32, addr_space="Shared")

    with nc.allow_low_precision("fp32r compute pipeline"), TileContext(nc) as tc:
        with (
            tc.tile_pool(name="const", bufs=1) as constp,
            tc.tile_pool(name="stage", bufs=3) as stagep,
            tc.tile_pool(name="mlps", bufs=2) as mlpp,
            tc.tile_pool(name="psconv", bufs=4, space="PSUM") as psconv,
            tc.tile_pool(name="psmlp", bufs=2, space="PSUM") as psmlp,
            tc.tile_pool(name="dram", bufs=1, space="DRAM") as dramp,
        ):
            wi_sb = constp.tile([128, 256], BF16, tag="wi")
            wf_sb = constp.tile([128, 256], BF16, tag="wf")
            wm_sb = constp.tile([82, P2], BF16, tag="wm")
            pooled = constp.tile([P2, C, P2], BF16, tag="pooled")
            s_dram = dramp.tile([P2, C, P2], F32)
            nc.sync.dma_start(out=wi_sb[:, :], in_=wi_d[:, :])
            nc.sync.dma_start(out=wf_sb[:, :], in_=wf_d[:, :])
            nc.sync.dma_start(out=wm_sb[:, :], in_=wm_d[:, :])

            def conv_mlp(src_d, w_sb, b_d, nlp, lp0, dst, dst_lp0):
                """conv+MLP+lrelu for `nlp` pairs starting at pair lp0 of
                src_d; writes (81, nlp, 128) into dst[:, dst_lp0:, :]."""
                st = stagep.tile([128, SUB_B, P2], BF16, tag="stage")
                nc.sync.dma_start(out=st[:, 0:nlp, :], in_=src_d[:, lp0:lp0 + nlp, :])
                ms = mlpp.tile([82, SUB_B, 128], BF16, tag="ms")
                nc.sync.dma_start(
                    out=ms[81:82, 0:nlp, :],
                    in_=b_d[:, 0:nlp * 128],
                )
                # conv: 2 pairs per PSUM bank; output (81, slot*64+c) per pair
                for pp in range(nlp // 2):
                    ps = psconv.tile([P2, 512], F32, tag="psc")
                    nc.tensor.matmul(
                        ps[0:P2, 0:256], _r(st[:, 2 * pp, :]), _r(w_sb[:, :]),
                        start=True, stop=True,
                    )
                    nc.tensor.matmul(
                        ps[0:P2, 256:512], _r(st[:, 2 * pp + 1, :]), _r(w_sb[:, :]),
                        start=True, stop=True,
                    )
                    # cols [0:128] and [256:384] hold the two pairs' results
                    src = ps[0:P2, 0:512].rearrange("p (a b) -> p a b", a=2)[:, :, 0:128]
                    d = ms[0:P2, 2 * pp:2 * pp + 2, :]
                    nc.vector.tensor_copy(d, src)
                # MLP (+bias row) and LeakyReLU
                ncols = nlp * 128
                flat = ms[:, 0:nlp, :].rearrange("p a b -> p (a b)")
                for q in range(ncols // 512):
                    mp = psmlp.tile([P2, 512], F32, tag="psm")
                    nc.tensor.matmul(
                        mp[0:P2, :], _r(wm_sb[:, :]), _r(flat[:, 512 * q:512 * (q + 1)]),
                        start=True, stop=True,
                    )
                    dcols = dst[0:P2, dst_lp0:dst_lp0 + nlp, :].rearrange(
                        "p a b -> p (a b)"
                    )[:, 512 * q:512 * (q + 1)]
                    nc.scalar.activation(
                        dcols, mp[0:P2, :],
                        mybir.ActivationFunctionType.Prelu, alpha=0.2,
                    )

            # ---------------- pass A: fea (y) + pooling partial sums --------
            with (
                tc.tile_pool(name="feaout", bufs=2) as feap,
                tc.tile_pool(name="ssbp", bufs=1) as ssbp,
                tc.tile_pool(name="comb", bufs=2) as combp,
            ):
                s_sb = ssbp.tile([P2, C, P2], F32, tag="ssb")
                for ch in range(NCH_A):
                    fea = feap.tile([P2, NLP_A, 128], F32, tag="fea")
                    conv_mlp(yp_d, wf_sb, bf_d, NLP_A, ch * NLP_A, fea, 0)
                    for g in range(NLP_A // 8):
                        sidx = ch * (NLP_A // 8) + g
                        nc.vector.tensor_reduce(
                            s_sb[0:P2, :, sidx:sidx + 1],
                            fea[0:P2, 8 * g:8 * (g + 1), :].rearrange(
                                "p l (s c) -> p c l s", s=2
                            ),
                            mybir.AxisListType.XY,
                            mybir.AluOpType.add,
                        )
                nc.gpsimd.dma_start(out=s_dram[:, :, :], in_=s_sb[:, :, :])
                nc.gpsimd.collective_compute(
                    "AllGather",
                    mybir.AluOpType.bypass,
                    replica_groups=[list(range(NCORES))],
                    ins=[s_dram[:, :, :]],
                    outs=[gath_d[:, :, :, :]],
                )
                # combine 648 global 16-sub-blocks into 81 bins of 8
                for cc in range(8):
                    tcb = combp.tile([P2, 8, NCORES * P2], F32, tag="tcb")
                    for k in range(NCORES):
                        nc.gpsimd.dma_start(
                            out=tcb[0:P2, :, P2 * k:P2 * (k + 1)],
                            in_=gath_d[k, :, 8 * cc:8 * (cc + 1), :],
                        )
                    pr = combp.tile([P2, 8, P2], F32, tag="pr")
                    nc.vector.tensor_reduce(
                        pr[0:P2, :, :],
                        tcb[0:P2, :, :].rearrange("p c (j m) -> p c j m", m=8),
                        mybir.AxisListType.X,
                        mybir.AluOpType.add,
                    )
                    nc.vector.tensor_scalar_mul(
                        pooled[0:P2, 8 * cc:8 * (cc + 1), :], pr[0:P2, :, :],
                        1.0 / 128.0,
                    )

            # ---------------- pass B: img (x) + attention -------------------
            with (
                tc.tile_pool(name="imgring", bufs=1) as imgp,
                tc.tile_pool(name="attev", bufs=4) as attevp,
                tc.tile_pool(name="psatt", bufs=2, space="PSUM") as psatt,
            ):
                for ring in range(NRING):
                    img = imgp.tile([P2, RING_B, 128], BF16, tag="img")
                    for sub in range(NSUB):
                        conv_mlp(
                            xp_d, wi_sb, bi_d, SUB_B,
                            ring * RING_B + sub * SUB_B, img, sub * SUB_B,
                        )
                    rhs = img[0:P2, :, :].rearrange("p l (s c) -> p c l s", s=2)
                    l0 = ring * RING_B * 2
                    ncols = RING_B * 2
                    for c in range(C):
                        ap = psatt.tile([P2, 512], F32, tag="psa")
                        nc.tensor.matmul(
                            ap[0:P2, 0:ncols],
                            _r(pooled[:, c:c + 1, :]),
                            _r(rhs[:, c:c + 1, :, :]),
                            start=True, stop=True,
                        )
                        ev = attevp.tile([P2, RING_B * 2], F32, tag="attev")
                        nc.scalar.copy(ev[0:P2, :], ap[0:P2, 0:ncols])
                        nc.sync.dma_start(
                            out=out_d[0:P2, c:c + 1, l0:l0 + ncols], in_=ev[0:P2, :]
                        )
    nc.compile()
    return nc


def _host_prep(x, y, w_img, b_img, w_fea, b_fea, w1, w2):
    f32 = np.float32
    weff = (w2.astype(np.float64) @ w1.astype(np.float64))  # (81, 81)
    wm = np.concatenate([weff.T, weff.sum(axis=1)[None, :]], axis=0).astype(f32)

    def pairw(w):
        blk = np.zeros((128, 128), dtype=f32)
        blk[0:64, 0:64] = w.T
        blk[64:128, 64:128] = w.T
        return np.concatenate([blk, blk], axis=1)

    wi = pairw(w_img.astype(f32))
    wf = pairw(w_fea.astype(f32))
    bi = np.tile(np.concatenate([b_img, b_img]).astype(f32), NLP_A)[None, :]
    bf = np.tile(np.concatenate([b_fea, b_fea]).astype(f32), NLP_A)[None, :]

    def unf_pairs(t):  # t: (1, 64, 72, 108, 108) -> list of (128, 648, 81)
        u = np.ascontiguousarray(
            t.reshape(C, NCORES, P, HW // P, P).transpose(1, 0, 3, 2, 4)
        )  # (8, 64, 1296, 9, 9) -> per core u[k]: (64, 1296, 81)
        u = u.reshape(NCORES, C, HW // P, P2)
        out = []
        for k in range(NCORES):
            v = u[k].reshape(C, LP, 2, P2).transpose(2, 0, 1, 3)  # (2, 64, 648, 81)
            out.append(np.ascontiguousarray(v.reshape(128, LP, P2)))
        return out

    bf16 = ml_dtypes.bfloat16
    xps = [a.astype(bf16) for a in unf_pairs(np.asarray(x, dtype=f32))]
    yps = [a.astype(bf16) for a in unf_pairs(np.asarray(y, dtype=f32))]
    shared = {"wi": wi.astype(bf16), "wf": wf.astype(bf16), "wm": wm.astype(bf16),
              "bi": bi.astype(bf16), "bf": bf.astype(bf16)}
    return [dict(shared, xp=xps[k], yp=yps[k]) for k in range(NCORES)]


def kernel(x, y, w_img, b_img, w_fea, b_fea, w1, w2):
    if "nc" not in _cache:
        _cache["nc"] = _build_nc()
    nc = _cache["nc"]
    in_maps = _host_prep(x, y, w_img, b_img, w_fea, b_fea, w1, w2)
    trace = bool(os.environ.get("KERNEL_TRACE"))
    res = run_bass_kernel_spmd(
        nc, in_maps, list(range(NCORES)), trace=trace
    )
    _cache["last_result"] = res
    out = np.empty((1, C, D, H, W), dtype=np.float32)
    ov = out.reshape(C, D, HW)
    for k in range(NCORES):
        att = res.results[k]["out"].transpose(1, 2, 0)  # (64, 1296, 81)
        blk = att.reshape(C, HW // P, P, P).transpose(0, 2, 1, 3).reshape(C, P, HW)
        ov[:, P * k:P * (k + 1), :] = blk
    return out
